# revision 5
# baseline (speedup 1.0000x reference)
"""Trainium2 Bass kernel for nn_ContradictionDetector (GNN edge message passing).

Reference computation (N=50000 nodes, D=128, H=256, E=400000 edges):
    pair   = concat(emb[src], emb[dst])                       # (E, 2D)
    scores = sigmoid(relu(pair @ Wd1 + bd1) @ Wd2 + bd2)      # (E,)
    resol  = relu(pair @ Wr1 + br1) @ Wr2 + br2               # (E, D)
    last-write-wins scatter of resol rows onto nodes (by edge order), blend.

Strategy: shard edges across 8 cores (50k each). Each core gathers the
src/dst embedding rows for its edges via dma_gather (int16 indices; edges
are pre-bucketed on the host by src/dst node-id half so every index fits a
32768-row table window), runs both MLPs with fp32 matmuls
(features-on-partitions layout via PE transposes), and writes per-edge
scores + resolution tiles (transposed layout). The tiny O(E)/O(N)
scatter-max merge and final blend run on the host.
"""

import os
import sys

for _p in ("/opt/trn_rl_repo",):
    if _p not in sys.path and os.path.isdir(_p):
        sys.path.insert(0, _p)

import numpy as np

import concourse.bass as bass
import concourse.tile as tile
from concourse import bacc, mybir
from concourse import bass_utils

F32 = mybir.dt.float32
I16 = mybir.dt.int16

N_NODES = 50000
D = 128
H = 256
N_CORES = 8
EDGE_TILE = 512       # edges per matmul tile (one fp32 moving-dim)
XTILE = 1024          # edges per dma_gather call (= 2 matmul tiles)
WINDOW = 32768        # dma_gather int16 index range -> table window rows


def build_nc(bases, n_nodes: int = N_NODES, enable_asserts: bool = False):
    """Build the per-core Bass program.

    bases: sequence of (src_base, dst_base) row offsets into the embedding
    table, one per 1024-edge x-tile.  All indices of an x-tile must fall in
    [base, base+WINDOW).
    """
    nx = len(bases)
    nt = 2 * nx
    window = min(WINDOW, n_nodes)
    nc = bacc.Bacc(
        "TRN2",
        target_bir_lowering=False,
        debug=False,
        enable_asserts=enable_asserts,
        num_devices=1,
    )

    emb = nc.dram_tensor("emb", [n_nodes, D], F32, kind="ExternalInput")
    sidx = nc.dram_tensor("sidx", [128, nx, 64], I16, kind="ExternalInput")
    didx = nc.dram_tensor("didx", [128, nx, 64], I16, kind="ExternalInput")
    wd1 = nc.dram_tensor("wd1", [128, 2, H], F32, kind="ExternalInput")
    wr1 = nc.dram_tensor("wr1", [128, 2, H], F32, kind="ExternalInput")
    wr2 = nc.dram_tensor("wr2", [128, 2, D], F32, kind="ExternalInput")
    wd2 = nc.dram_tensor("wd2", [128, 2], F32, kind="ExternalInput")
    bd1 = nc.dram_tensor("bd1", [128, 2], F32, kind="ExternalInput")
    br1 = nc.dram_tensor("br1", [128, 2], F32, kind="ExternalInput")
    br2 = nc.dram_tensor("br2", [128, 1], F32, kind="ExternalInput")
    bd2 = nc.dram_tensor("bd2", [1, 1], F32, kind="ExternalInput")
    ident = nc.dram_tensor("ident", [128, 128], F32, kind="ExternalInput")

    resT = nc.dram_tensor("resT", [nt, 128, EDGE_TILE], F32, kind="ExternalOutput")
    scr = nc.dram_tensor("scr", [nt, EDGE_TILE], F32, kind="ExternalOutput")

    with tile.TileContext(nc) as tc:
        with (
            tc.tile_pool(name="const", bufs=1) as cpool,
            tc.tile_pool(name="xp", bufs=2) as xpool,
            tc.tile_pool(name="sp", bufs=3) as spool,
            tc.tile_pool(name="hp", bufs=2) as hpool,
            tc.tile_pool(name="ps_xt", bufs=2, space="PSUM") as ps_xt,
            tc.tile_pool(name="ps_h1", bufs=4, space="PSUM") as ps_h1,
            tc.tile_pool(name="ps_out", bufs=1, space="PSUM") as ps_out,
        ):
            sidx_sb = cpool.tile([128, nx, 64], I16)
            nc.sync.dma_start(sidx_sb[:], sidx.ap())
            didx_sb = cpool.tile([128, nx, 64], I16)
            nc.sync.dma_start(didx_sb[:], didx.ap())
            wd1_sb = cpool.tile([128, 2, H], F32)
            nc.sync.dma_start(wd1_sb[:], wd1.ap())
            wr1_sb = cpool.tile([128, 2, H], F32)
            nc.sync.dma_start(wr1_sb[:], wr1.ap())
            wr2_sb = cpool.tile([128, 2, D], F32)
            nc.sync.dma_start(wr2_sb[:], wr2.ap())
            wd2_sb = cpool.tile([128, 2], F32)
            nc.sync.dma_start(wd2_sb[:], wd2.ap())
            bd1_sb = cpool.tile([128, 2], F32)
            nc.sync.dma_start(bd1_sb[:], bd1.ap())
            br1_sb = cpool.tile([128, 2], F32)
            nc.sync.dma_start(br1_sb[:], br1.ap())
            br2_sb = cpool.tile([128, 1], F32)
            nc.sync.dma_start(br2_sb[:], br2.ap())
            bd2_sb = cpool.tile([1, 1], F32)
            nc.sync.dma_start(bd2_sb[:], bd2.ap())
            ident_sb = cpool.tile([128, 128], F32)
            nc.sync.dma_start(ident_sb[:], ident.ap())

            for xt in range(nx):
                sb, db = bases[xt]
                # Gather 1024 src rows and 1024 dst rows.
                # Layout: xs[p, c, :] = emb[sb + sidx_flat[c*128+p], :]
                xs = xpool.tile([128, 8, D], F32, tag="xs")
                nc.gpsimd.dma_gather(
                    out_ap=xs[:],
                    in_ap=emb.ap()[sb : sb + window],
                    idxs_ap=sidx_sb[:, xt, :],
                    num_idxs=XTILE,
                    num_idxs_reg=XTILE,
                    elem_size=D,
                )
                xd = xpool.tile([128, 8, D], F32, tag="xd")
                nc.gpsimd.dma_gather(
                    out_ap=xd[:],
                    in_ap=emb.ap()[db : db + window],
                    idxs_ap=didx_sb[:, xt, :],
                    num_idxs=XTILE,
                    num_idxs_reg=XTILE,
                    elem_size=D,
                )

                for h in range(2):
                    t = 2 * xt + h
                    # Transpose to feature-major: [128 feat, 512 edges]
                    xsT_ps = ps_xt.tile([128, EDGE_TILE], F32, tag="xt")
                    xdT_ps = ps_xt.tile([128, EDGE_TILE], F32, tag="xt")
                    for j in range(4):
                        nc.tensor.transpose(
                            xsT_ps[:, j * 128 : (j + 1) * 128],
                            xs[:, 4 * h + j, :],
                            ident_sb[:],
                        )
                    for j in range(4):
                        nc.tensor.transpose(
                            xdT_ps[:, j * 128 : (j + 1) * 128],
                            xd[:, 4 * h + j, :],
                            ident_sb[:],
                        )
                    xsT = spool.tile([128, EDGE_TILE], F32, tag="xsT")
                    xdT = spool.tile([128, EDGE_TILE], F32, tag="xdT")
                    nc.vector.tensor_copy(xsT[:], xsT_ps[:])
                    nc.vector.tensor_copy(xdT[:], xdT_ps[:])

                    # Layer 1 of both MLPs: h1T[p, mc, e] = relu(W1.T pair + b1)
                    h1dT = hpool.tile([128, 2, EDGE_TILE], F32, tag="h1d")
                    h1rT = hpool.tile([128, 2, EDGE_TILE], F32, tag="h1r")
                    for w_sb, b_sb, hT in (
                        (wd1_sb, bd1_sb, h1dT),
                        (wr1_sb, br1_sb, h1rT),
                    ):
                        for mc in range(2):
                            ps = ps_h1.tile([128, EDGE_TILE], F32, tag="h1ps")
                            nc.tensor.matmul(
                                ps[:],
                                w_sb[:, 0, mc * 128 : (mc + 1) * 128],
                                xsT[:],
                                start=True,
                                stop=False,
                            )
                            nc.tensor.matmul(
                                ps[:],
                                w_sb[:, 1, mc * 128 : (mc + 1) * 128],
                                xdT[:],
                                start=False,
                                stop=True,
                            )
                            nc.scalar.activation(
                                hT[:, mc, :],
                                ps[:],
                                mybir.ActivationFunctionType.Relu,
                                bias=b_sb[:, mc : mc + 1],
                            )

                    # Layer 2 resolver: resolution.T = Wr2.T @ h1r + br2
                    rps = ps_out.tile([128, EDGE_TILE], F32, tag="rps")
                    nc.tensor.matmul(
                        rps[:], wr2_sb[:, 0, :], h1rT[:, 0, :], start=True, stop=False
                    )
                    nc.tensor.matmul(
                        rps[:], wr2_sb[:, 1, :], h1rT[:, 1, :], start=False, stop=True
                    )
                    rt = spool.tile([128, EDGE_TILE], F32, tag="rt")
                    nc.vector.tensor_tensor(
                        out=rt[:],
                        in0=rps[:],
                        in1=br2_sb[:, 0:1].to_broadcast([128, EDGE_TILE]),
                        op=mybir.AluOpType.add,
                    )
                    nc.sync.dma_start(resT.ap()[t], rt[:])

                    # Layer 2 detector -> sigmoid -> [1, 512]
                    sps = ps_out.tile([1, EDGE_TILE], F32, tag="sps")
                    nc.tensor.matmul(
                        sps[:], wd2_sb[:, 0:1], h1dT[:, 0, :], start=True, stop=False
                    )
                    nc.tensor.matmul(
                        sps[:], wd2_sb[:, 1:2], h1dT[:, 1, :], start=False, stop=True
                    )
                    sct = spool.tile([1, EDGE_TILE], F32, tag="sct")
                    nc.scalar.activation(
                        sct[:],
                        sps[:],
                        mybir.ActivationFunctionType.Sigmoid,
                        bias=bd2_sb[0:1, 0:1],
                    )
                    nc.sync.dma_start(scr.ap()[t : t + 1, :], sct[:])

    nc.compile()
    return nc


def _prep_weights(Wd1, bd1, Wd2, bd2, Wr1, br1, Wr2, br2):
    f32 = np.float32
    return {
        "wd1": np.ascontiguousarray(
            np.asarray(Wd1, f32).reshape(2, 128, H).transpose(1, 0, 2)
        ),
        "wr1": np.ascontiguousarray(
            np.asarray(Wr1, f32).reshape(2, 128, H).transpose(1, 0, 2)
        ),
        "wr2": np.ascontiguousarray(
            np.asarray(Wr2, f32).reshape(2, 128, D).transpose(1, 0, 2)
        ),
        "wd2": np.ascontiguousarray(np.asarray(Wd2, f32).reshape(2, 128).T),
        "bd1": np.ascontiguousarray(np.asarray(bd1, f32).reshape(2, 128).T),
        "br1": np.ascontiguousarray(np.asarray(br1, f32).reshape(2, 128).T),
        "br2": np.ascontiguousarray(np.asarray(br2, f32).reshape(128, 1)),
        "bd2": np.asarray(bd2, f32).reshape(1, 1),
        "ident": np.eye(128, dtype=f32),
    }


def _wrap_idx(flat: np.ndarray, nx: int) -> np.ndarray:
    """[nx*1024] int16 -> [128, nx, 64]: idx[q, xt, m] = flat[xt*1024 + m*16 + q%16]."""
    w = flat.reshape(nx, 64, 16)           # [xt, m, q]
    w = w.transpose(2, 0, 1)               # [16, xt, 64]
    return np.ascontiguousarray(np.tile(w, (8, 1, 1)))  # [128, nx, 64]


def _bucketize(src: np.ndarray, dst: np.ndarray, hi_base: int):
    """Group one core's edges by (src-half, dst-half). Returns
    (perm, counts[4]) with stable in-bucket order."""
    b = (src >= WINDOW).astype(np.int8) * 2 + (dst >= WINDOW)
    perm = np.argsort(b, kind="stable")
    counts = np.bincount(b, minlength=4)
    return perm, counts


_NC_CACHE: dict = {}
LAST_RESULTS = None  # BassKernelResults of the most recent device run


def _get_nc(bases, n_nodes: int = N_NODES):
    key = (tuple(bases), n_nodes)
    if key not in _NC_CACHE:
        _NC_CACHE[key] = build_nc(bases, n_nodes)
    return _NC_CACHE[key]


def kernel(
    embeddings,
    edge_index,
    Wd1,
    bd1,
    Wd2,
    bd2,
    Wr1,
    br1,
    Wr2,
    br2,
    _trace: bool = False,
):
    global LAST_RESULTS
    emb = np.ascontiguousarray(np.asarray(embeddings, np.float32))
    eidx = np.asarray(edge_index)
    n_nodes = emb.shape[0]
    hi_base = max(n_nodes - WINDOW, 0)
    e_total = eidx.shape[1]
    ec = e_total // N_CORES

    src_all = eidx[0].astype(np.int64)
    dst_all = eidx[1].astype(np.int64)

    # --- host-side bucketing (indices must fit int16 table windows) ---
    perms, counts = [], []
    for c in range(N_CORES):
        sl = slice(c * ec, (c + 1) * ec)
        p, cnt = _bucketize(src_all[sl], dst_all[sl], hi_base)
        perms.append(p)
        counts.append(cnt)
    counts = np.stack(counts)  # [8, 4]
    nx_b = [int(-(-counts[:, b].max() // XTILE)) for b in range(4)]
    nx = sum(nx_b)
    bases = []
    for b in range(4):
        sb = hi_base if (b >> 1) else 0
        db = hi_base if (b & 1) else 0
        bases.extend([(sb, db)] * nx_b[b])

    nc = _get_nc(bases, n_nodes)

    common = _prep_weights(Wd1, bd1, Wd2, bd2, Wr1, br1, Wr2, br2)
    common["emb"] = emb

    in_maps = []
    pos_list = []
    for c in range(N_CORES):
        sl = slice(c * ec, (c + 1) * ec)
        src, dst = src_all[sl], dst_all[sl]
        perm, cnt = perms[c], counts[c]
        s_flat = np.zeros(nx * XTILE, np.int64)
        d_flat = np.zeros(nx * XTILE, np.int64)
        pos = np.empty(ec, np.int64)
        off = 0
        pstart = 0
        for b in range(4):
            k = int(cnt[b])
            sel = perm[pstart : pstart + k]
            sb = hi_base if (b >> 1) else 0
            db = hi_base if (b & 1) else 0
            s_flat[off : off + k] = src[sel] - sb
            d_flat[off : off + k] = dst[sel] - db
            pos[sel] = off + np.arange(k)
            off += nx_b[b] * XTILE
            pstart += k
        m = dict(common)
        m["sidx"] = _wrap_idx(s_flat.astype(np.int16), nx)
        m["didx"] = _wrap_idx(d_flat.astype(np.int16), nx)
        in_maps.append(m)
        pos_list.append(pos)

    tmpdir = None
    if _trace:
        tmpdir = "/root/problem/profdir"
        import shutil

        shutil.rmtree(tmpdir, ignore_errors=True)
        os.makedirs(tmpdir, exist_ok=True)
    res = bass_utils.run_bass_kernel_spmd(
        nc, in_maps, core_ids=list(range(N_CORES)), trace=_trace, tmpdir=tmpdir
    )
    LAST_RESULTS = res

    # ---- host-side merge (O(E) + O(N*D)) ----
    pos_all = np.stack(pos_list)  # [8, ec] padded position of each edge
    scr_all = np.stack([r["scr"].reshape(-1) for r in res.results])  # [8, nt*512]
    scores = scr_all[np.arange(N_CORES)[:, None], pos_all].reshape(-1).astype(
        np.float32
    )

    mask = scores > 0.5
    order = np.where(mask, np.arange(1, e_total + 1, dtype=np.int64), 0)
    last = np.zeros(n_nodes, np.int64)
    np.maximum.at(last, src_all, order)
    np.maximum.at(last, dst_all, order)

    sel = last > 0
    win = last[sel] - 1  # global winning edge id per touched node
    core = win // ec
    q = pos_all[core, win % ec]  # padded position within core
    t = q // EDGE_TILE
    col = q % EDGE_TILE

    resT_all = np.stack([r["resT"] for r in res.results])  # [8, nt, 128, 512]
    rrows = resT_all[core, t, :, col]  # [nsel, 128]

    resolved = emb.copy()
    resolved[sel] = (emb[sel] + rrows) * np.float32(0.5)
    return resolved, scores


# revision 6
# speedup vs baseline: 1.3423x; 1.3423x over previous
"""Trainium2 Bass kernel for nn_ContradictionDetector (GNN edge message passing).

Reference computation (N=50000 nodes, D=128, H=256, E=400000 edges):
    pair   = concat(emb[src], emb[dst])                       # (E, 2D)
    scores = sigmoid(relu(pair @ Wd1 + bd1) @ Wd2 + bd2)      # (E,)
    resol  = relu(pair @ Wr1 + br1) @ Wr2 + br2               # (E, D)
    last-write-wins scatter of resol rows onto nodes (by edge order), blend.

Strategy: shard edges across 8 cores (50k each). Each core gathers the
src/dst embedding rows for its edges via dma_gather (int16 indices; edges
are pre-bucketed on the host by src/dst node-id half so every index fits a
32768-row table window), runs both MLPs with fp32 matmuls
(features-on-partitions layout via PE transposes), and writes per-edge
scores + resolution tiles (transposed layout). The tiny O(E)/O(N)
scatter-max merge and final blend run on the host.
"""

import os
import sys

for _p in ("/opt/trn_rl_repo",):
    if _p not in sys.path and os.path.isdir(_p):
        sys.path.insert(0, _p)

import ml_dtypes
import numpy as np

import concourse.bass as bass
import concourse.tile as tile
from concourse import bacc, mybir
from concourse import bass_utils

F32 = mybir.dt.float32
BF16 = mybir.dt.bfloat16
I16 = mybir.dt.int16

N_NODES = 50000
D = 128
H = 256
N_CORES = 8
EDGE_TILE = 512       # edges per matmul tile (one fp32 moving-dim)
XTILE = 1024          # edges per dma_gather call (= 2 matmul tiles)
WINDOW = 32768        # dma_gather int16 index range -> table window rows


def build_nc(bases, n_nodes: int = N_NODES, enable_asserts: bool = False):
    """Build the per-core Bass program.

    bases: sequence of (src_base, dst_base) row offsets into the embedding
    table, one per 1024-edge x-tile.  All indices of an x-tile must fall in
    [base, base+WINDOW).
    """
    nx = len(bases)
    nt = 2 * nx
    window = min(WINDOW, n_nodes)
    nc = bacc.Bacc(
        "TRN2",
        target_bir_lowering=False,
        debug=False,
        enable_asserts=enable_asserts,
        num_devices=1,
    )

    emb = nc.dram_tensor("emb", [n_nodes, D], F32, kind="ExternalInput")
    sidx = nc.dram_tensor("sidx", [128, nx, 64], I16, kind="ExternalInput")
    didx = nc.dram_tensor("didx", [128, nx, 64], I16, kind="ExternalInput")
    wd1 = nc.dram_tensor("wd1", [128, 2, H], F32, kind="ExternalInput")
    wr1 = nc.dram_tensor("wr1", [128, 2, H], BF16, kind="ExternalInput")
    wr2 = nc.dram_tensor("wr2", [128, 2, D], BF16, kind="ExternalInput")
    wd2 = nc.dram_tensor("wd2", [128, 2], F32, kind="ExternalInput")
    bd1 = nc.dram_tensor("bd1", [128, 2], F32, kind="ExternalInput")
    br1 = nc.dram_tensor("br1", [128, 2], F32, kind="ExternalInput")
    br2 = nc.dram_tensor("br2", [128, 1], F32, kind="ExternalInput")
    bd2 = nc.dram_tensor("bd2", [1, 1], F32, kind="ExternalInput")
    ident = nc.dram_tensor("ident", [128, 128], F32, kind="ExternalInput")

    resT = nc.dram_tensor("resT", [nt, 128, EDGE_TILE], BF16, kind="ExternalOutput")
    scr = nc.dram_tensor("scr", [nt, EDGE_TILE], F32, kind="ExternalOutput")

    with tile.TileContext(nc) as tc:
        with (
            tc.tile_pool(name="const", bufs=1) as cpool,
            tc.tile_pool(name="xp", bufs=2) as xpool,
            tc.tile_pool(name="sp", bufs=3) as spool,
            tc.tile_pool(name="hp", bufs=2) as hpool,
            tc.tile_pool(name="ps_xt", bufs=2, space="PSUM") as ps_xt,
            tc.tile_pool(name="ps_h1", bufs=4, space="PSUM") as ps_h1,
            tc.tile_pool(name="ps_out", bufs=1, space="PSUM") as ps_out,
        ):
            sidx_sb = cpool.tile([128, nx, 64], I16)
            nc.sync.dma_start(sidx_sb[:], sidx.ap())
            didx_sb = cpool.tile([128, nx, 64], I16)
            nc.sync.dma_start(didx_sb[:], didx.ap())
            wd1_sb = cpool.tile([128, 2, H], F32)
            nc.sync.dma_start(wd1_sb[:], wd1.ap())
            wr1_sb = cpool.tile([128, 2, H], BF16)
            nc.sync.dma_start(wr1_sb[:], wr1.ap())
            wr2_sb = cpool.tile([128, 2, D], BF16)
            nc.sync.dma_start(wr2_sb[:], wr2.ap())
            wd2_sb = cpool.tile([128, 2], F32)
            nc.sync.dma_start(wd2_sb[:], wd2.ap())
            bd1_sb = cpool.tile([128, 2], F32)
            nc.sync.dma_start(bd1_sb[:], bd1.ap())
            br1_sb = cpool.tile([128, 2], F32)
            nc.sync.dma_start(br1_sb[:], br1.ap())
            br2_sb = cpool.tile([128, 1], F32)
            nc.sync.dma_start(br2_sb[:], br2.ap())
            bd2_sb = cpool.tile([1, 1], F32)
            nc.sync.dma_start(bd2_sb[:], bd2.ap())
            ident_sb = cpool.tile([128, 128], F32)
            nc.sync.dma_start(ident_sb[:], ident.ap())

            for xt in range(nx):
                sb, db = bases[xt]
                # Gather 1024 src rows and 1024 dst rows.
                # Layout: xs[p, c, :] = emb[sb + sidx_flat[c*128+p], :]
                xs = xpool.tile([128, 8, D], F32, tag="xs")
                nc.gpsimd.dma_gather(
                    out_ap=xs[:],
                    in_ap=emb.ap()[sb : sb + window],
                    idxs_ap=sidx_sb[:, xt, :],
                    num_idxs=XTILE,
                    num_idxs_reg=XTILE,
                    elem_size=D,
                )
                xd = xpool.tile([128, 8, D], F32, tag="xd")
                nc.gpsimd.dma_gather(
                    out_ap=xd[:],
                    in_ap=emb.ap()[db : db + window],
                    idxs_ap=didx_sb[:, xt, :],
                    num_idxs=XTILE,
                    num_idxs_reg=XTILE,
                    elem_size=D,
                )

                for h in range(2):
                    t = 2 * xt + h
                    # Transpose to feature-major: [128 feat, 512 edges]
                    xsT_ps = ps_xt.tile([128, EDGE_TILE], F32, tag="xt")
                    xdT_ps = ps_xt.tile([128, EDGE_TILE], F32, tag="xt")
                    for j in range(4):
                        nc.tensor.transpose(
                            xsT_ps[:, j * 128 : (j + 1) * 128],
                            xs[:, 4 * h + j, :],
                            ident_sb[:],
                        )
                    for j in range(4):
                        nc.tensor.transpose(
                            xdT_ps[:, j * 128 : (j + 1) * 128],
                            xd[:, 4 * h + j, :],
                            ident_sb[:],
                        )
                    xsT = spool.tile([128, EDGE_TILE], F32, tag="xsT")
                    xdT = spool.tile([128, EDGE_TILE], F32, tag="xdT")
                    nc.vector.tensor_copy(xsT[:], xsT_ps[:])
                    nc.vector.tensor_copy(xdT[:], xdT_ps[:])
                    xsTb = spool.tile([128, EDGE_TILE], BF16, tag="xsTb")
                    xdTb = spool.tile([128, EDGE_TILE], BF16, tag="xdTb")
                    nc.scalar.copy(xsTb[:], xsT_ps[:])
                    nc.scalar.copy(xdTb[:], xdT_ps[:])

                    # Layer 1 of both MLPs: h1T[p, mc, e] = relu(W1.T pair + b1)
                    h1dT = hpool.tile([128, 2, EDGE_TILE], F32, tag="h1d")
                    h1rT = hpool.tile([128, 2, EDGE_TILE], BF16, tag="h1r")
                    for w_sb, b_sb, hT, rs, rd in (
                        (wd1_sb, bd1_sb, h1dT, xsT, xdT),
                        (wr1_sb, br1_sb, h1rT, xsTb, xdTb),
                    ):
                        for mc in range(2):
                            ps = ps_h1.tile([128, EDGE_TILE], F32, tag="h1ps")
                            nc.tensor.matmul(
                                ps[:],
                                w_sb[:, 0, mc * 128 : (mc + 1) * 128],
                                rs[:],
                                start=True,
                                stop=False,
                            )
                            nc.tensor.matmul(
                                ps[:],
                                w_sb[:, 1, mc * 128 : (mc + 1) * 128],
                                rd[:],
                                start=False,
                                stop=True,
                            )
                            nc.scalar.activation(
                                hT[:, mc, :],
                                ps[:],
                                mybir.ActivationFunctionType.Relu,
                                bias=b_sb[:, mc : mc + 1],
                            )

                    # Layer 2 resolver: resolution.T = Wr2.T @ h1r + br2
                    rps = ps_out.tile([128, EDGE_TILE], F32, tag="rps")
                    nc.tensor.matmul(
                        rps[:], wr2_sb[:, 0, :], h1rT[:, 0, :], start=True, stop=False
                    )
                    nc.tensor.matmul(
                        rps[:], wr2_sb[:, 1, :], h1rT[:, 1, :], start=False, stop=True
                    )
                    rt = spool.tile([128, EDGE_TILE], BF16, tag="rt")
                    nc.vector.tensor_tensor(
                        out=rt[:],
                        in0=rps[:],
                        in1=br2_sb[:, 0:1].to_broadcast([128, EDGE_TILE]),
                        op=mybir.AluOpType.add,
                    )
                    nc.sync.dma_start(resT.ap()[t], rt[:])

                    # Layer 2 detector -> sigmoid -> [1, 512]
                    sps = ps_out.tile([1, EDGE_TILE], F32, tag="sps")
                    nc.tensor.matmul(
                        sps[:], wd2_sb[:, 0:1], h1dT[:, 0, :], start=True, stop=False
                    )
                    nc.tensor.matmul(
                        sps[:], wd2_sb[:, 1:2], h1dT[:, 1, :], start=False, stop=True
                    )
                    sct = spool.tile([1, EDGE_TILE], F32, tag="sct")
                    nc.scalar.activation(
                        sct[:],
                        sps[:],
                        mybir.ActivationFunctionType.Sigmoid,
                        bias=bd2_sb[0:1, 0:1],
                    )
                    nc.sync.dma_start(scr.ap()[t : t + 1, :], sct[:])

    nc.compile()
    return nc


def _prep_weights(Wd1, bd1, Wd2, bd2, Wr1, br1, Wr2, br2):
    f32 = np.float32
    return {
        "wd1": np.ascontiguousarray(
            np.asarray(Wd1, f32).reshape(2, 128, H).transpose(1, 0, 2)
        ),
        "wr1": np.ascontiguousarray(
            np.asarray(Wr1, f32).reshape(2, 128, H).transpose(1, 0, 2)
        ).astype(ml_dtypes.bfloat16),
        "wr2": np.ascontiguousarray(
            np.asarray(Wr2, f32).reshape(2, 128, D).transpose(1, 0, 2)
        ).astype(ml_dtypes.bfloat16),
        "wd2": np.ascontiguousarray(np.asarray(Wd2, f32).reshape(2, 128).T),
        "bd1": np.ascontiguousarray(np.asarray(bd1, f32).reshape(2, 128).T),
        "br1": np.ascontiguousarray(np.asarray(br1, f32).reshape(2, 128).T),
        "br2": np.ascontiguousarray(np.asarray(br2, f32).reshape(128, 1)),
        "bd2": np.asarray(bd2, f32).reshape(1, 1),
        "ident": np.eye(128, dtype=f32),
    }


def _wrap_idx(flat: np.ndarray, nx: int) -> np.ndarray:
    """[nx*1024] int16 -> [128, nx, 64]: idx[q, xt, m] = flat[xt*1024 + m*16 + q%16]."""
    w = flat.reshape(nx, 64, 16)           # [xt, m, q]
    w = w.transpose(2, 0, 1)               # [16, xt, 64]
    return np.ascontiguousarray(np.tile(w, (8, 1, 1)))  # [128, nx, 64]


def _bucketize(src: np.ndarray, dst: np.ndarray, hi_base: int):
    """Group one core's edges by (src-half, dst-half). Returns
    (perm, counts[4]) with stable in-bucket order."""
    b = (src >= WINDOW).astype(np.int8) * 2 + (dst >= WINDOW)
    perm = np.argsort(b, kind="stable")
    counts = np.bincount(b, minlength=4)
    return perm, counts


_NC_CACHE: dict = {}
LAST_RESULTS = None  # BassKernelResults of the most recent device run


def _get_nc(bases, n_nodes: int = N_NODES):
    key = (tuple(bases), n_nodes)
    if key not in _NC_CACHE:
        _NC_CACHE[key] = build_nc(bases, n_nodes)
    return _NC_CACHE[key]


def kernel(
    embeddings,
    edge_index,
    Wd1,
    bd1,
    Wd2,
    bd2,
    Wr1,
    br1,
    Wr2,
    br2,
    _trace: bool = False,
):
    global LAST_RESULTS
    emb = np.ascontiguousarray(np.asarray(embeddings, np.float32))
    eidx = np.asarray(edge_index)
    n_nodes = emb.shape[0]
    hi_base = max(n_nodes - WINDOW, 0)
    e_total = eidx.shape[1]
    ec = e_total // N_CORES

    src_all = eidx[0].astype(np.int64)
    dst_all = eidx[1].astype(np.int64)

    # --- host-side bucketing (indices must fit int16 table windows) ---
    perms, counts = [], []
    for c in range(N_CORES):
        sl = slice(c * ec, (c + 1) * ec)
        p, cnt = _bucketize(src_all[sl], dst_all[sl], hi_base)
        perms.append(p)
        counts.append(cnt)
    counts = np.stack(counts)  # [8, 4]
    nx_b = [int(-(-counts[:, b].max() // XTILE)) for b in range(4)]
    nx = sum(nx_b)
    bases = []
    for b in range(4):
        sb = hi_base if (b >> 1) else 0
        db = hi_base if (b & 1) else 0
        bases.extend([(sb, db)] * nx_b[b])

    nc = _get_nc(bases, n_nodes)

    common = _prep_weights(Wd1, bd1, Wd2, bd2, Wr1, br1, Wr2, br2)
    common["emb"] = emb

    in_maps = []
    pos_list = []
    for c in range(N_CORES):
        sl = slice(c * ec, (c + 1) * ec)
        src, dst = src_all[sl], dst_all[sl]
        perm, cnt = perms[c], counts[c]
        s_flat = np.zeros(nx * XTILE, np.int64)
        d_flat = np.zeros(nx * XTILE, np.int64)
        pos = np.empty(ec, np.int64)
        off = 0
        pstart = 0
        for b in range(4):
            k = int(cnt[b])
            sel = perm[pstart : pstart + k]
            sb = hi_base if (b >> 1) else 0
            db = hi_base if (b & 1) else 0
            s_flat[off : off + k] = src[sel] - sb
            d_flat[off : off + k] = dst[sel] - db
            pos[sel] = off + np.arange(k)
            off += nx_b[b] * XTILE
            pstart += k
        m = dict(common)
        m["sidx"] = _wrap_idx(s_flat.astype(np.int16), nx)
        m["didx"] = _wrap_idx(d_flat.astype(np.int16), nx)
        in_maps.append(m)
        pos_list.append(pos)

    tmpdir = None
    if _trace:
        tmpdir = "/root/problem/profdir"
        import shutil

        shutil.rmtree(tmpdir, ignore_errors=True)
        os.makedirs(tmpdir, exist_ok=True)
    res = bass_utils.run_bass_kernel_spmd(
        nc, in_maps, core_ids=list(range(N_CORES)), trace=_trace, tmpdir=tmpdir
    )
    LAST_RESULTS = res

    # ---- host-side merge (O(E) + O(N*D)) ----
    pos_all = np.stack(pos_list)  # [8, ec] padded position of each edge
    scr_all = np.stack([r["scr"].reshape(-1) for r in res.results])  # [8, nt*512]
    scores = scr_all[np.arange(N_CORES)[:, None], pos_all].reshape(-1).astype(
        np.float32
    )

    mask = scores > 0.5
    order = np.where(mask, np.arange(1, e_total + 1, dtype=np.int64), 0)
    last = np.zeros(n_nodes, np.int64)
    np.maximum.at(last, src_all, order)
    np.maximum.at(last, dst_all, order)

    sel = last > 0
    win = last[sel] - 1  # global winning edge id per touched node
    core = win // ec
    q = pos_all[core, win % ec]  # padded position within core
    t = q // EDGE_TILE
    col = q % EDGE_TILE

    resT_all = np.stack([r["resT"] for r in res.results])  # [8, nt, 128, 512]
    rrows = resT_all[core, t, :, col].astype(np.float32)  # [nsel, 128]

    resolved = emb.copy()
    resolved[sel] = (emb[sel] + rrows) * np.float32(0.5)
    return resolved, scores


# revision 9
# speedup vs baseline: 1.3860x; 1.0326x over previous
"""Trainium2 Bass kernel for nn_ContradictionDetector (GNN edge message passing).

Reference computation (N=50000 nodes, D=128, H=256, E=400000 edges):
    pair   = concat(emb[src], emb[dst])                       # (E, 2D)
    scores = sigmoid(relu(pair @ Wd1 + bd1) @ Wd2 + bd2)      # (E,)
    resol  = relu(pair @ Wr1 + br1) @ Wr2 + br2               # (E, D)
    last-write-wins scatter of resol rows onto nodes (by edge order), blend.

Strategy: shard edges across 8 cores (50k each). Each core gathers the
src/dst embedding rows for its edges via dma_gather (int16 indices; edges
are pre-bucketed on the host by src/dst node-id half so every index fits a
32768-row table window), runs both MLPs with fp32 matmuls
(features-on-partitions layout via PE transposes), and writes per-edge
scores + resolution tiles (transposed layout). The tiny O(E)/O(N)
scatter-max merge and final blend run on the host.
"""

import os
import sys

for _p in ("/opt/trn_rl_repo",):
    if _p not in sys.path and os.path.isdir(_p):
        sys.path.insert(0, _p)

import ml_dtypes
import numpy as np

import concourse.bass as bass
import concourse.tile as tile
from concourse import bacc, mybir
from concourse import bass_utils

F32 = mybir.dt.float32
BF16 = mybir.dt.bfloat16
I16 = mybir.dt.int16

N_NODES = 50000
D = 128
H = 256
N_CORES = 8
EDGE_TILE = 512       # edges per matmul tile (one fp32 moving-dim)
XTILE = 1024          # edges per dma_gather call (= 2 matmul tiles)
WINDOW = 32768        # dma_gather int16 index range -> table window rows


def build_nc(bases, n_nodes: int = N_NODES, enable_asserts: bool = False):
    """Build the per-core Bass program.

    bases: sequence of (src_base, dst_base) row offsets into the embedding
    table, one per 1024-edge x-tile.  All indices of an x-tile must fall in
    [base, base+WINDOW).
    """
    nx = len(bases)
    nt = 2 * nx
    window = min(WINDOW, n_nodes)
    nc = bacc.Bacc(
        "TRN2",
        target_bir_lowering=False,
        debug=False,
        enable_asserts=enable_asserts,
        num_devices=1,
    )

    emb = nc.dram_tensor("emb", [n_nodes, D], F32, kind="ExternalInput")
    sidx = nc.dram_tensor("sidx", [128, nx, 64], I16, kind="ExternalInput")
    didx = nc.dram_tensor("didx", [128, nx, 64], I16, kind="ExternalInput")
    wd1 = nc.dram_tensor("wd1", [128, 2, H], F32, kind="ExternalInput")
    wr1 = nc.dram_tensor("wr1", [128, 2, H], BF16, kind="ExternalInput")
    wr2 = nc.dram_tensor("wr2", [128, 2, D], BF16, kind="ExternalInput")
    wd2 = nc.dram_tensor("wd2", [128, 2], F32, kind="ExternalInput")
    bd1 = nc.dram_tensor("bd1", [128, 2], F32, kind="ExternalInput")
    br1 = nc.dram_tensor("br1", [128, 2], F32, kind="ExternalInput")
    br2 = nc.dram_tensor("br2", [128, 1], F32, kind="ExternalInput")
    bd2 = nc.dram_tensor("bd2", [1, 1], F32, kind="ExternalInput")
    ident = nc.dram_tensor("ident", [128, 128], F32, kind="ExternalInput")

    resT = nc.dram_tensor("resT", [nt, 128, EDGE_TILE], BF16, kind="ExternalOutput")
    scr = nc.dram_tensor("scr", [nt, EDGE_TILE], F32, kind="ExternalOutput")

    with tile.TileContext(nc) as tc:
        with (
            tc.tile_pool(name="const", bufs=1) as cpool,
            tc.tile_pool(name="xp", bufs=3) as xpool,
            tc.tile_pool(name="sp", bufs=3) as spool,
            tc.tile_pool(name="hp", bufs=2) as hpool,
            tc.tile_pool(name="ps_xt", bufs=2, space="PSUM") as ps_xt,
            tc.tile_pool(name="ps_h1", bufs=4, space="PSUM") as ps_h1,
            tc.tile_pool(name="ps_out", bufs=1, space="PSUM") as ps_out,
        ):
            sidx_sb = cpool.tile([128, nx, 64], I16)
            nc.sync.dma_start(sidx_sb[:], sidx.ap())
            didx_sb = cpool.tile([128, nx, 64], I16)
            nc.sync.dma_start(didx_sb[:], didx.ap())
            wd1_sb = cpool.tile([128, 2, H], F32)
            nc.sync.dma_start(wd1_sb[:], wd1.ap())
            wr1_sb = cpool.tile([128, 2, H], BF16)
            nc.sync.dma_start(wr1_sb[:], wr1.ap())
            wr2_sb = cpool.tile([128, 2, D], BF16)
            nc.sync.dma_start(wr2_sb[:], wr2.ap())
            wd2_sb = cpool.tile([128, 2], F32)
            nc.sync.dma_start(wd2_sb[:], wd2.ap())
            bd1_sb = cpool.tile([128, 2], F32)
            nc.sync.dma_start(bd1_sb[:], bd1.ap())
            br1_sb = cpool.tile([128, 2], F32)
            nc.sync.dma_start(br1_sb[:], br1.ap())
            br2_sb = cpool.tile([128, 1], F32)
            nc.sync.dma_start(br2_sb[:], br2.ap())
            bd2_sb = cpool.tile([1, 1], F32)
            nc.sync.dma_start(bd2_sb[:], bd2.ap())
            ident_sb = cpool.tile([128, 128], F32)
            nc.sync.dma_start(ident_sb[:], ident.ap())

            for xt in range(nx):
                sb, db = bases[xt]
                # Gather 1024 src rows and 1024 dst rows.
                # Layout: xs[p, c, :] = emb[sb + sidx_flat[c*128+p], :]
                xs = xpool.tile([128, 8, D], F32, tag="xs")
                nc.gpsimd.dma_gather(
                    out_ap=xs[:],
                    in_ap=emb.ap()[sb : sb + window],
                    idxs_ap=sidx_sb[:, xt, :],
                    num_idxs=XTILE,
                    num_idxs_reg=XTILE,
                    elem_size=D,
                )
                xd = xpool.tile([128, 8, D], F32, tag="xd")
                nc.gpsimd.dma_gather(
                    out_ap=xd[:],
                    in_ap=emb.ap()[db : db + window],
                    idxs_ap=didx_sb[:, xt, :],
                    num_idxs=XTILE,
                    num_idxs_reg=XTILE,
                    elem_size=D,
                )

                for h in range(2):
                    t = 2 * xt + h
                    # Transpose to feature-major: [128 feat, 512 edges]
                    xsT_ps = ps_xt.tile([128, EDGE_TILE], F32, tag="xt")
                    xdT_ps = ps_xt.tile([128, EDGE_TILE], F32, tag="xt")
                    for j in range(4):
                        nc.tensor.transpose(
                            xsT_ps[:, j * 128 : (j + 1) * 128],
                            xs[:, 4 * h + j, :],
                            ident_sb[:],
                        )
                    for j in range(4):
                        nc.tensor.transpose(
                            xdT_ps[:, j * 128 : (j + 1) * 128],
                            xd[:, 4 * h + j, :],
                            ident_sb[:],
                        )
                    xsT = spool.tile([128, EDGE_TILE], F32, tag="xsT")
                    xdT = spool.tile([128, EDGE_TILE], F32, tag="xdT")
                    nc.vector.tensor_copy(xsT[:], xsT_ps[:])
                    nc.vector.tensor_copy(xdT[:], xdT_ps[:])
                    xsTb = spool.tile([128, EDGE_TILE], BF16, tag="xsTb")
                    xdTb = spool.tile([128, EDGE_TILE], BF16, tag="xdTb")
                    nc.scalar.copy(xsTb[:], xsT_ps[:])
                    nc.scalar.copy(xdTb[:], xdT_ps[:])

                    # Layer 1 of both MLPs: h1T[p, mc, e] = relu(W1.T pair + b1)
                    h1dT = hpool.tile([128, 2, EDGE_TILE], F32, tag="h1d")
                    h1rT = hpool.tile([128, 2, EDGE_TILE], BF16, tag="h1r")
                    for w_sb, b_sb, hT, rs, rd in (
                        (wd1_sb, bd1_sb, h1dT, xsT, xdT),
                        (wr1_sb, br1_sb, h1rT, xsTb, xdTb),
                    ):
                        for mc in range(2):
                            ps = ps_h1.tile([128, EDGE_TILE], F32, tag="h1ps")
                            nc.tensor.matmul(
                                ps[:],
                                w_sb[:, 0, mc * 128 : (mc + 1) * 128],
                                rs[:],
                                start=True,
                                stop=False,
                            )
                            nc.tensor.matmul(
                                ps[:],
                                w_sb[:, 1, mc * 128 : (mc + 1) * 128],
                                rd[:],
                                start=False,
                                stop=True,
                            )
                            nc.scalar.activation(
                                hT[:, mc, :],
                                ps[:],
                                mybir.ActivationFunctionType.Relu,
                                bias=b_sb[:, mc : mc + 1],
                            )

                    # Layer 2 resolver: resolution.T = Wr2.T @ h1r + br2
                    rps = ps_out.tile([128, EDGE_TILE], F32, tag="rps")
                    nc.tensor.matmul(
                        rps[:], wr2_sb[:, 0, :], h1rT[:, 0, :], start=True, stop=False
                    )
                    nc.tensor.matmul(
                        rps[:], wr2_sb[:, 1, :], h1rT[:, 1, :], start=False, stop=True
                    )
                    rt = spool.tile([128, EDGE_TILE], BF16, tag="rt")
                    nc.vector.tensor_tensor(
                        out=rt[:],
                        in0=rps[:],
                        in1=br2_sb[:, 0:1].to_broadcast([128, EDGE_TILE]),
                        op=mybir.AluOpType.add,
                    )
                    nc.sync.dma_start(resT.ap()[t], rt[:])

                    # Layer 2 detector -> sigmoid -> [1, 512]
                    sps = ps_out.tile([1, EDGE_TILE], F32, tag="sps")
                    nc.tensor.matmul(
                        sps[:], wd2_sb[:, 0:1], h1dT[:, 0, :], start=True, stop=False
                    )
                    nc.tensor.matmul(
                        sps[:], wd2_sb[:, 1:2], h1dT[:, 1, :], start=False, stop=True
                    )
                    sct = spool.tile([1, EDGE_TILE], F32, tag="sct")
                    nc.scalar.activation(
                        sct[:],
                        sps[:],
                        mybir.ActivationFunctionType.Sigmoid,
                        bias=bd2_sb[0:1, 0:1],
                    )
                    nc.sync.dma_start(scr.ap()[t : t + 1, :], sct[:])

    nc.compile()
    return nc


def _prep_weights(Wd1, bd1, Wd2, bd2, Wr1, br1, Wr2, br2):
    f32 = np.float32
    return {
        "wd1": np.ascontiguousarray(
            np.asarray(Wd1, f32).reshape(2, 128, H).transpose(1, 0, 2)
        ),
        "wr1": np.ascontiguousarray(
            np.asarray(Wr1, f32).reshape(2, 128, H).transpose(1, 0, 2)
        ).astype(ml_dtypes.bfloat16),
        "wr2": np.ascontiguousarray(
            np.asarray(Wr2, f32).reshape(2, 128, D).transpose(1, 0, 2)
        ).astype(ml_dtypes.bfloat16),
        "wd2": np.ascontiguousarray(np.asarray(Wd2, f32).reshape(2, 128).T),
        "bd1": np.ascontiguousarray(np.asarray(bd1, f32).reshape(2, 128).T),
        "br1": np.ascontiguousarray(np.asarray(br1, f32).reshape(2, 128).T),
        "br2": np.ascontiguousarray(np.asarray(br2, f32).reshape(128, 1)),
        "bd2": np.asarray(bd2, f32).reshape(1, 1),
        "ident": np.eye(128, dtype=f32),
    }


def _wrap_idx(flat: np.ndarray, nx: int) -> np.ndarray:
    """[nx*1024] int16 -> [128, nx, 64]: idx[q, xt, m] = flat[xt*1024 + m*16 + q%16]."""
    w = flat.reshape(nx, 64, 16)           # [xt, m, q]
    w = w.transpose(2, 0, 1)               # [16, xt, 64]
    return np.ascontiguousarray(np.tile(w, (8, 1, 1)))  # [128, nx, 64]


def _bucketize(src: np.ndarray, dst: np.ndarray, hi_base: int):
    """Group one core's edges by (src-half, dst-half). Returns
    (perm, counts[4]) with stable in-bucket order."""
    b = (src >= WINDOW).astype(np.int8) * 2 + (dst >= WINDOW)
    perm = np.argsort(b, kind="stable")
    counts = np.bincount(b, minlength=4)
    return perm, counts


_NC_CACHE: dict = {}
LAST_RESULTS = None  # BassKernelResults of the most recent device run


def _get_nc(bases, n_nodes: int = N_NODES):
    key = (tuple(bases), n_nodes)
    if key not in _NC_CACHE:
        _NC_CACHE[key] = build_nc(bases, n_nodes)
    return _NC_CACHE[key]


def kernel(
    embeddings,
    edge_index,
    Wd1,
    bd1,
    Wd2,
    bd2,
    Wr1,
    br1,
    Wr2,
    br2,
    _trace: bool = False,
):
    global LAST_RESULTS
    emb = np.ascontiguousarray(np.asarray(embeddings, np.float32))
    eidx = np.asarray(edge_index)
    n_nodes = emb.shape[0]
    hi_base = max(n_nodes - WINDOW, 0)
    e_total = eidx.shape[1]
    ec = e_total // N_CORES

    src_all = eidx[0].astype(np.int64)
    dst_all = eidx[1].astype(np.int64)

    # --- host-side bucketing (indices must fit int16 table windows) ---
    perms, counts = [], []
    for c in range(N_CORES):
        sl = slice(c * ec, (c + 1) * ec)
        p, cnt = _bucketize(src_all[sl], dst_all[sl], hi_base)
        perms.append(p)
        counts.append(cnt)
    counts = np.stack(counts)  # [8, 4]
    nx_b = [int(-(-counts[:, b].max() // XTILE)) for b in range(4)]
    nx = sum(nx_b)
    bases = []
    for b in range(4):
        sb = hi_base if (b >> 1) else 0
        db = hi_base if (b & 1) else 0
        bases.extend([(sb, db)] * nx_b[b])

    nc = _get_nc(bases, n_nodes)

    common = _prep_weights(Wd1, bd1, Wd2, bd2, Wr1, br1, Wr2, br2)
    common["emb"] = emb

    in_maps = []
    pos_list = []
    for c in range(N_CORES):
        sl = slice(c * ec, (c + 1) * ec)
        src, dst = src_all[sl], dst_all[sl]
        perm, cnt = perms[c], counts[c]
        s_flat = np.zeros(nx * XTILE, np.int64)
        d_flat = np.zeros(nx * XTILE, np.int64)
        pos = np.empty(ec, np.int64)
        off = 0
        pstart = 0
        for b in range(4):
            k = int(cnt[b])
            sel = perm[pstart : pstart + k]
            sb = hi_base if (b >> 1) else 0
            db = hi_base if (b & 1) else 0
            s_flat[off : off + k] = src[sel] - sb
            d_flat[off : off + k] = dst[sel] - db
            pos[sel] = off + np.arange(k)
            off += nx_b[b] * XTILE
            pstart += k
        m = dict(common)
        m["sidx"] = _wrap_idx(s_flat.astype(np.int16), nx)
        m["didx"] = _wrap_idx(d_flat.astype(np.int16), nx)
        in_maps.append(m)
        pos_list.append(pos)

    tmpdir = None
    if _trace:
        tmpdir = "/root/problem/profdir"
        import shutil

        shutil.rmtree(tmpdir, ignore_errors=True)
        os.makedirs(tmpdir, exist_ok=True)
    res = bass_utils.run_bass_kernel_spmd(
        nc, in_maps, core_ids=list(range(N_CORES)), trace=_trace, tmpdir=tmpdir
    )
    LAST_RESULTS = res

    # ---- host-side merge (O(E) + O(N*D)) ----
    pos_all = np.stack(pos_list)  # [8, ec] padded position of each edge
    scr_all = np.stack([r["scr"].reshape(-1) for r in res.results])  # [8, nt*512]
    scores = scr_all[np.arange(N_CORES)[:, None], pos_all].reshape(-1).astype(
        np.float32
    )

    mask = scores > 0.5
    order = np.where(mask, np.arange(1, e_total + 1, dtype=np.int64), 0)
    last = np.zeros(n_nodes, np.int64)
    np.maximum.at(last, src_all, order)
    np.maximum.at(last, dst_all, order)

    sel = last > 0
    win = last[sel] - 1  # global winning edge id per touched node
    core = win // ec
    q = pos_all[core, win % ec]  # padded position within core
    t = q // EDGE_TILE
    col = q % EDGE_TILE

    resT_all = np.stack([r["resT"] for r in res.results])  # [8, nt, 128, 512]
    rrows = resT_all[core, t, :, col].astype(np.float32)  # [nsel, 128]

    resolved = emb.copy()
    resolved[sel] = (emb[sel] + rrows) * np.float32(0.5)
    return resolved, scores
